# revision 22
# baseline (speedup 1.0000x reference)
"""Trainium2 Bass kernel for CRANUnit retrieval-KNN module.

Math (exact algebraic restructure of the reference):
  scores[b,n]   = cache_keys[b,n,:] . x[b,:]
  top8 vals/idx = hardware Max8/MaxIndex8 per batch row
  w8            = softmax(top8 scores)          (folded into coeff scale)
  qk[b,:]       = x[b,:] @ M + qkb,  M = Wq.T @ Wk,  qkb = bq @ Wk
  logits[b,k,l] = (zones[b,k,l,:] . qk[b,:]) / sqrt(H)
     (the reference's extra  query.bk  term is constant over l and k for
      fixed b, so it cancels in the per-(b,k) softmax over l)
  attn          = softmax_l(logits)
  coeff[b,k,l]  = w8[b,k] * attn[b,k,l]
  attn_out[b]   = sum_{k,l} coeff[b,k,l] * zones[b,k,l,:]
  out[b]        = attn_out[b] @ Wc[:, :H].T + x[b] @ Wc[:, H:].T + bc

Sharding: batch B=32 split over 8 cores (4 rows each); weights replicated.
Only the top-8 zones per row are ever read from HBM (indirect DMA gather).
"""

import numpy as np

B, E, H, N, L, K = 32, 1024, 1024, 128, 64, 8
NCORES = 8
BS = B // NCORES  # batch rows per core = 4
NC_CHUNKS = H // 128  # 8 chunks of 128 along E/H
KL = K * L  # 512 gathered rows per batch row
ZCH = KL // 128  # 4 gather chunks of 128 rows
INV_SQRT_H = 1.0 / float(np.sqrt(H))

# dtype mode for the big matmuls: "f32r" (fast fp32) or "f32" (4x slower)
MATMUL_MODE = "f32r"


def _mm_cast(ap):
    return ap


def emit(nc, tc):
    """Emit the per-core program. Same program on all 8 cores (SPMD)."""
    from contextlib import ExitStack

    import concourse.bass as bass
    import concourse.mybir as mybir
    from concourse.masks import make_identity

    ctx = ExitStack()

    f32 = mybir.dt.float32
    f32r = mybir.dt.float32r
    u32 = mybir.dt.uint32

    # ---- DRAM parameters (order = binding order) ----
    x_d = nc.declare_dram_parameter("x_sh", [BS, E], f32, isOutput=False)
    xr_d = nc.declare_dram_parameter("x_shr", [BS, E], f32r, isOutput=False)
    keys_d = nc.declare_dram_parameter("keys_sh", [BS, N, E], f32, isOutput=False)
    vals_d = nc.declare_dram_parameter("vals_sh", [BS, N, L, H], f32r, isOutput=False)
    m_d = nc.declare_dram_parameter("m_w", [E, H], f32r, isOutput=False)
    qkb_d = nc.declare_dram_parameter("qkb", [1, H], f32, isOutput=False)
    wct_d = nc.declare_dram_parameter("wct", [2 * H, H], f32r, isOutput=False)
    bc_d = nc.declare_dram_parameter("bc", [1, H], f32, isOutput=False)
    out_d = nc.declare_dram_parameter("out_sh", [BS, H], f32, isOutput=True)

    vals_flat = vals_d.ap().rearrange("b n l h -> (b n l) h")

    sb = ctx.enter_context(tc.tile_pool(name="sb", bufs=1))
    dpool = ctx.enter_context(tc.tile_pool(name="dpool", bufs=2, space="DRAM"))
    kpool = ctx.enter_context(tc.tile_pool(name="kpool", bufs=2))
    wpool = ctx.enter_context(tc.tile_pool(name="wpool", bufs=3))
    zpool = ctx.enter_context(tc.tile_pool(name="zpool", bufs=1))
    scpool = ctx.enter_context(tc.tile_pool(name="scpool", bufs=2))
    bpool = ctx.enter_context(tc.tile_pool(name="bpool", bufs=2))
    pqk = ctx.enter_context(tc.tile_pool(name="pqk", bufs=1, space="PSUM"))
    ptr = ctx.enter_context(tc.tile_pool(name="ptr", bufs=1, space="PSUM"))
    patt = ctx.enter_context(tc.tile_pool(name="patt", bufs=1, space="PSUM"))
    pout = ctx.enter_context(tc.tile_pool(name="pout", bufs=1, space="PSUM"))

    # ---- constants / small loads ----
    ident128 = sb.tile([128, 128], f32, tag="ident128")
    make_identity(nc, ident128)
    ident4 = sb.tile([4, 4], f32, tag="ident4")
    make_identity(nc, ident4)

    iota_p = sb.tile([128, 1], u32, tag="iota_p")
    nc.gpsimd.iota(iota_p, pattern=[[0, 1]], base=0, channel_multiplier=1)
    sub64 = sb.tile([128, 1], u32, tag="sub64")
    nc.vector.memset(sub64[0:64, :], 0)
    nc.vector.memset(sub64[64:128, :], 64)
    l_mod = sb.tile([128, 1], u32, tag="l_mod")  # p % 64
    nc.vector.tensor_tensor(
        out=l_mod, in0=iota_p, in1=sub64, op=mybir.AluOpType.subtract
    )

    xrows = []
    for b in range(BS):
        xr = sb.tile([1, E], f32, tag=f"xr{b}")
        nc.sync.dma_start(out=xr, in_=x_d.ap()[b : b + 1, :])
        xrows.append(xr)
    xt = sb.tile([128, NC_CHUNKS, BS], f32r, tag="xt")  # x transposed chunks
    for c in range(NC_CHUNKS):
        nc.sync.dma_start(
            out=xt[:, c, :],
            in_=xr_d.ap()[:, c * 128 : (c + 1) * 128].rearrange("b p -> p b"),
        )

    bc_sb = sb.tile([1, H], f32, tag="bc_sb")
    nc.sync.dma_start(out=bc_sb, in_=bc_d.ap())
    bc_b = sb.tile([BS, H], f32, tag="bc_b")
    nc.gpsimd.partition_broadcast(bc_b, bc_sb[0:1, :])

    qkb_sb = sb.tile([1, H], f32, tag="qkb_sb")
    nc.sync.dma_start(out=qkb_sb, in_=qkb_d.ap())
    qkb_b = sb.tile([BS, H], f32, tag="qkb_b")
    nc.gpsimd.partition_broadcast(qkb_b, qkb_sb[0:1, :])

    # ---- cache-key loads (first priority on DMA queues) ----
    ktiles = []
    for b in range(BS):
        kb = kpool.tile([128, E], f32, tag="kb")
        nc.sync.dma_start(out=kb, in_=keys_d.ap()[b])
        ktiles.append(kb)

    # ---- broadcast x rows; scores via DVE mult+reduce ----
    scoresP = sb.tile([128, BS], f32, tag="scoresP")
    for b in range(BS):
        xb = bpool.tile([128, E], f32, tag="bc128")
        nc.gpsimd.partition_broadcast(xb, xrows[b])
        scr = scpool.tile([128, E], f32, tag="scr")
        nc.vector.tensor_tensor(
            out=scr, in0=ktiles[b], in1=xb, op=mybir.AluOpType.mult
        )
        nc.vector.tensor_reduce(
            out=scoresP[:, b : b + 1],
            in_=scr,
            axis=mybir.AxisListType.X,
            op=mybir.AluOpType.add,
        )

    # ---- final projection, x-half (PE can start as soon as wct streams) ----
    psum_out = pout.tile([BS, H], f32, tag="psum_out", space="PSUM")
    for j in range(NC_CHUNKS):
        wch = wpool.tile([128, H], f32r, tag="w")
        nc.sync.dma_start(out=wch, in_=wct_d.ap()[H + j * 128 : H + (j + 1) * 128, :])
        for hh in range(2):
            nc.tensor.matmul(
                out=psum_out[:, hh * 512 : (hh + 1) * 512],
                lhsT=_mm_cast(xt[:, j, :]),
                rhs=_mm_cast(wch[:, hh * 512 : (hh + 1) * 512]),
                start=(j == 0),
                stop=False,
                skip_group_check=True,
            )

    # ---- qk = x @ M + qkb ----
    psum_qk = pqk.tile([BS, H], f32, tag="psum_qk", space="PSUM")
    for c in range(NC_CHUNKS):
        mch = wpool.tile([128, H], f32r, tag="w")
        nc.sync.dma_start(out=mch, in_=m_d.ap()[c * 128 : (c + 1) * 128, :])
        for hh in range(2):
            nc.tensor.matmul(
                out=psum_qk[:, hh * 512 : (hh + 1) * 512],
                lhsT=_mm_cast(xt[:, c, :]),
                rhs=_mm_cast(mch[:, hh * 512 : (hh + 1) * 512]),
                start=(c == 0),
                stop=(c == NC_CHUNKS - 1),
            )
    qk_sb = sb.tile([BS, H], f32, tag="qk_sb")
    nc.vector.tensor_tensor(
        out=qk_sb, in0=psum_qk, in1=qkb_b, op=mybir.AluOpType.add
    )

    # ---- transpose scores -> [BS, N]; top-8 ----
    psum_tr = ptr.tile([BS, 128], f32, tag="psum_tr", space="PSUM")
    nc.tensor.transpose(out=psum_tr, in_=scoresP, identity=ident128)
    scoresF = sb.tile([BS, N], f32, tag="scoresF")
    nc.vector.tensor_copy(out=scoresF, in_=psum_tr)

    top8 = sb.tile([BS, 8], f32, tag="top8")
    topi = sb.tile([BS, 8], u32, tag="topi")
    nc.vector.max_with_indices(out_max=top8, out_indices=topi, in_=scoresF)

    negmax = sb.tile([BS, 1], f32, tag="negmax")
    nc.vector.tensor_scalar_mul(negmax, top8[:, 0:1], -1.0)
    e8 = sb.tile([BS, 8], f32, tag="e8")
    sum8 = sb.tile([BS, 1], f32, tag="sum8")
    nc.scalar.activation(
        out=e8,
        in_=top8,
        func=mybir.ActivationFunctionType.Exp,
        bias=negmax,
        scale=1.0,
        accum_out=sum8,
    )

    # ---- gather offsets + indirect zone gathers ----
    offs = []
    for b in range(BS):
        trow = sb.tile([1, 8], u32, tag=f"trow{b}")
        nc.sync.dma_start(out=trow, in_=topi[b : b + 1, :])
        t8u = bpool.tile([128, 8], u32, tag=f"t8u{b % 2}")
        nc.gpsimd.partition_broadcast(t8u, trow)
        t8v = t8u.rearrange("p (k two) -> p k two", two=2)
        off_b = sb.tile([128, ZCH], u32, tag=f"off{b}")
        nc.vector.tensor_copy(out=off_b[0:64, :], in_=t8v[0:64, :, 0])
        nc.vector.tensor_copy(out=off_b[64:128, :], in_=t8v[64:128, :, 1])
        nc.vector.tensor_scalar(
            out=off_b,
            in0=off_b,
            scalar1=L,  # slot index n -> row n*L
            scalar2=b * N * L,  # batch-row base in vals_flat
            op0=mybir.AluOpType.mult,
            op1=mybir.AluOpType.add,
        )
        nc.vector.tensor_tensor(
            out=off_b,
            in0=off_b,
            in1=l_mod.to_broadcast([128, ZCH]),
            op=mybir.AluOpType.add,
        )
        offs.append(off_b)

    ztiles = {}
    for b in range(BS):
        for c in range(ZCH):
            z = zpool.tile([128, H], f32r, tag=f"z{b}_{c}")
            nc.gpsimd.indirect_dma_start(
                out=z,
                out_offset=None,
                in_=vals_flat,
                in_offset=bass.IndirectOffsetOnAxis(ap=offs[b][:, c : c + 1], axis=0),
            )
            ztiles[(b, c)] = z

    # ---- logits via DVE mult+reduce against broadcast qk ----
    logitsP = sb.tile([128, BS * ZCH], f32, tag="logitsP")
    for b in range(BS):
        qrow = sb.tile([1, H], f32, tag=f"qrow{b % 2}")
        nc.sync.dma_start(out=qrow, in_=qk_sb[b : b + 1, :])
        qkb_bc = bpool.tile([128, H], f32, tag="bc128")
        nc.gpsimd.partition_broadcast(qkb_bc, qrow)
        for c in range(ZCH):
            scr2 = scpool.tile([128, H], f32, tag="scr")
            nc.vector.tensor_tensor(
                out=scr2,
                in0=ztiles[(b, c)].bitcast(f32),
                in1=qkb_bc,
                op=mybir.AluOpType.mult,
            )
            nc.vector.tensor_reduce(
                out=logitsP[:, b * ZCH + c : b * ZCH + c + 1],
                in_=scr2,
                axis=mybir.AxisListType.X,
                op=mybir.AluOpType.add,
            )

    # ---- logits layout fix: [128, BS*ZCH] -> [BS, KL] via DRAM bounce ----
    lgt = sb.tile([BS, KL], f32, tag="lgt")
    for b in range(BS):
        dscr = dpool.tile([128, ZCH], f32, tag=f"dscr{b % 2}")
        nc.sync.dma_start(out=dscr, in_=logitsP[:, b * ZCH : (b + 1) * ZCH])
        nc.sync.dma_start(
            out=lgt[b : b + 1, :],
            in_=dscr.rearrange("p c -> c p"),
        )

    # ---- zone softmax + combined coefficients ----
    lgt_v = lgt.rearrange("b (k l) -> b k l", k=K)
    zmax = sb.tile([BS, K], f32, tag="zmax")
    nc.vector.tensor_reduce(
        out=zmax, in_=lgt_v, axis=mybir.AxisListType.X, op=mybir.AluOpType.max
    )
    lc = sb.tile([BS, KL], f32, tag="lc")
    nc.vector.tensor_tensor(
        out=lc.rearrange("b (k l) -> b k l", k=K),
        in0=lgt_v,
        in1=zmax[:, :, None].to_broadcast([BS, K, L]),
        op=mybir.AluOpType.subtract,
    )
    ez = sb.tile([BS, KL], f32, tag="ez")
    nc.scalar.activation(out=ez, in_=lc, func=mybir.ActivationFunctionType.Exp)
    zsum = sb.tile([BS, K], f32, tag="zsum")
    nc.vector.tensor_reduce(
        out=zsum,
        in_=ez.rearrange("b (k l) -> b k l", k=K),
        axis=mybir.AxisListType.X,
        op=mybir.AluOpType.add,
    )
    t8s = sb.tile([BS, K], f32, tag="t8s")
    nc.vector.tensor_scalar_mul(t8s, zsum, sum8)
    rec8 = sb.tile([BS, K], f32, tag="rec8")
    nc.vector.reciprocal(rec8, t8s)
    s8 = sb.tile([BS, K], f32, tag="s8")
    nc.vector.tensor_tensor(out=s8, in0=e8, in1=rec8, op=mybir.AluOpType.mult)
    coeff = sb.tile([BS, KL], f32, tag="coeff")
    nc.vector.tensor_tensor(
        out=coeff.rearrange("b (k l) -> b k l", k=K),
        in0=ez.rearrange("b (k l) -> b k l", k=K),
        in1=s8[:, :, None].to_broadcast([BS, K, L]),
        op=mybir.AluOpType.mult,
    )

    # ---- transpose coeff -> [128, ZCH*BS] column layout ----
    psum_ct = ptr.tile([128, ZCH * BS], f32, tag="psum_ct", space="PSUM")
    for c in range(ZCH):
        nc.tensor.transpose(
            out=psum_ct[:, c * BS : (c + 1) * BS],
            in_=coeff[:, c * 128 : (c + 1) * 128],
            identity=ident4,
        )
    ctT = sb.tile([128, ZCH * BS], f32r, tag="ctT")
    nc.vector.tensor_copy(out=ctT, in_=psum_ct)

    # ---- attn_out[b] = coeff_b.T @ Z_b ----
    attT = sb.tile([128, NC_CHUNKS, BS], f32r, tag="attT")
    for b in range(BS):
        psum_att = patt.tile([1, H], f32, tag="psum_att", space="PSUM")
        for c in range(ZCH):
            for hh in range(2):
                nc.tensor.matmul(
                    out=psum_att[:, hh * 512 : (hh + 1) * 512],
                    lhsT=_mm_cast(ctT[:, c * BS + b : c * BS + b + 1]),
                    rhs=_mm_cast(ztiles[(b, c)][:, hh * 512 : (hh + 1) * 512]),
                    start=(c == 0),
                    stop=(c == ZCH - 1),
                )
        att_sb = sb.tile([1, H], f32r, tag=f"att_sb{b % 2}")
        nc.scalar.copy(out=att_sb, in_=psum_att)
        datt = dpool.tile([128, NC_CHUNKS], f32r, tag=f"datt{b % 2}")
        nc.sync.dma_start(out=datt.rearrange("p c -> c p"), in_=att_sb)
        nc.sync.dma_start(out=attT[:, :, b], in_=datt)

    # ---- final projection, attn half; bias; store ----
    for j in range(NC_CHUNKS):
        wch = wpool.tile([128, H], f32r, tag="w")
        nc.sync.dma_start(out=wch, in_=wct_d.ap()[j * 128 : (j + 1) * 128, :])
        for hh in range(2):
            nc.tensor.matmul(
                out=psum_out[:, hh * 512 : (hh + 1) * 512],
                lhsT=_mm_cast(attT[:, j, :]),
                rhs=_mm_cast(wch[:, hh * 512 : (hh + 1) * 512]),
                start=False,
                stop=(j == NC_CHUNKS - 1),
                skip_group_check=True,
            )
    out_sb = sb.tile([BS, H], f32, tag="out_sb")
    nc.vector.tensor_tensor(
        out=out_sb, in0=psum_out, in1=bc_b, op=mybir.AluOpType.add
    )
    nc.sync.dma_start(out=out_d.ap(), in_=out_sb)
    ctx.close()


def build_program():
    import concourse.bacc as bacc
    import concourse.tile as tile

    nc = bacc.Bacc("TRN2", target_bir_lowering=False, debug=False, enable_asserts=False)
    with tile.TileContext(nc) as tc:
        emit(nc, tc)
    nc.compile()
    return nc


def host_prep(inputs, Wq, bq, Wk, bk, Wc, bc, cache_keys, cache_values):
    """Host-side weight folding + per-core input shards."""
    x = np.asarray(inputs, dtype=np.float32)
    Wq = np.asarray(Wq, dtype=np.float32)
    Wk = np.asarray(Wk, dtype=np.float32)
    Wc = np.asarray(Wc, dtype=np.float32)
    # fold the attention 1/sqrt(H) into the query projection
    m_w = np.ascontiguousarray((Wq.T @ Wk) * INV_SQRT_H, dtype=np.float32)  # [E, H]
    qkb = np.ascontiguousarray(
        ((np.asarray(bq, np.float32) @ Wk) * INV_SQRT_H).reshape(1, H),
        dtype=np.float32,
    )
    wct = np.ascontiguousarray(Wc.T, dtype=np.float32)  # [2H, H]
    bc2 = np.ascontiguousarray(np.asarray(bc, np.float32).reshape(1, H))
    keys = np.asarray(cache_keys, dtype=np.float32)
    vals = np.asarray(cache_values, dtype=np.float32)

    in_maps = []
    for c in range(NCORES):
        sl = slice(c * BS, (c + 1) * BS)
        in_maps.append(
            {
                "x_sh": np.ascontiguousarray(x[sl]),
                "x_shr": np.ascontiguousarray(x[sl]),
                "keys_sh": np.ascontiguousarray(keys[sl]),
                "vals_sh": np.ascontiguousarray(vals[sl]),
                "m_w": m_w,
                "qkb": qkb,
                "wct": wct,
                "bc": bc2,
            }
        )
    return in_maps


_CACHED = {}


def kernel(**inputs):
    from concourse.bass_utils import run_bass_kernel_spmd

    in_maps = host_prep(**inputs)
    if "nc" not in _CACHED:
        _CACHED["nc"] = build_program()
    nc = _CACHED["nc"]
    res = run_bass_kernel_spmd(nc, in_maps, core_ids=list(range(NCORES)))
    _CACHED["last_res"] = res
    out = np.concatenate([r["out_sh"] for r in res.results], axis=0)
    return out.astype(np.float32)


if __name__ == "__main__":
    import reference

    ins = {k: np.asarray(v) for k, v in reference.setup_inputs().items()}
    got = kernel(**ins)
    exp = np.asarray(reference.reference(**ins))
    err = np.abs(got - exp).max() / (np.abs(exp).max() + 1e-12)
    print("Relative error:", err)


# revision 25
# speedup vs baseline: 1.4916x; 1.4916x over previous
"""Trainium2 Bass kernel for CRANUnit retrieval-KNN module.

Math (exact algebraic restructure of the reference):
  scores[b,n]   = cache_keys[b,n,:] . x[b,:]
  top8 vals/idx = hardware Max8/MaxIndex8 per batch row
  w8            = softmax(top8 scores)          (folded into coeff scale)
  qk[b,:]       = (x[b,:] @ M + qkb) / sqrt(H),  M = Wq.T @ Wk,  qkb = bq @ Wk
  logits[b,k,l] = zones[b,k,l,:] . qk[b,:]
     (the reference's extra  query.bk  term is constant over l for fixed
      (b,k), so it cancels in the per-(b,k) softmax over l)
  attn          = softmax_l(logits)
  coeff[b,k,l]  = w8[b,k] * attn[b,k,l]
  attn_out[b]   = sum_{k,l} coeff[b,k,l] * zones[b,k,l,:]
  out[b]        = attn_out[b] @ Wc[:, :H].T + x[b] @ Wc[:, H:].T + bc

Sharding: batch B=32 split over 8 cores (4 rows each); weights replicated.
Only the top-8 zones per row are ever read from HBM (indirect DMA gather).
All partition-crossing layout fixups run on the PE (strided-column
transposes); big weight streams are spread across 4 DMA queues.
"""

import numpy as np

B, E, H, N, L, K = 32, 1024, 1024, 128, 64, 8
NCORES = 8
BS = B // NCORES  # batch rows per core = 4
NC_CHUNKS = H // 128  # 8 chunks of 128 along E/H
KL = K * L  # 512 gathered rows per batch row
ZCH = KL // 128  # 4 gather chunks of 128 rows
INV_SQRT_H = 1.0 / float(np.sqrt(H))


def emit(nc, tc):
    """Emit the per-core program. Same program on all 8 cores (SPMD)."""
    from contextlib import ExitStack

    import concourse.bass as bass
    import concourse.mybir as mybir
    from concourse.masks import make_identity

    ctx = ExitStack()

    f32 = mybir.dt.float32
    f32r = mybir.dt.float32r
    u32 = mybir.dt.uint32

    # ---- DRAM parameters ----
    x_d = nc.declare_dram_parameter("x_sh", [BS, E], f32, isOutput=False)
    keys_d = nc.declare_dram_parameter("keys_sh", [BS, N, E], f32, isOutput=False)
    vals_d = nc.declare_dram_parameter("vals_sh", [BS, N, L, H], f32r, isOutput=False)
    m_d = nc.declare_dram_parameter("m_w", [E, H], f32r, isOutput=False)
    qkb_d = nc.declare_dram_parameter("qkb", [1, H], f32, isOutput=False)
    wct_d = nc.declare_dram_parameter("wct", [2 * H, H], f32r, isOutput=False)
    bc_d = nc.declare_dram_parameter("bc", [1, H], f32, isOutput=False)
    out_d = nc.declare_dram_parameter("out_sh", [BS, H], f32, isOutput=True)

    vals_flat = vals_d.ap().rearrange("b n l h -> (b n l) h")

    sb = ctx.enter_context(tc.tile_pool(name="sb", bufs=1))
    kpool = ctx.enter_context(tc.tile_pool(name="kpool", bufs=2))
    wpool = ctx.enter_context(tc.tile_pool(name="wpool", bufs=4))
    zpool = ctx.enter_context(tc.tile_pool(name="zpool", bufs=1))
    scpool = ctx.enter_context(tc.tile_pool(name="scpool", bufs=2))
    bpool = ctx.enter_context(tc.tile_pool(name="bpool", bufs=2))
    ptr = ctx.enter_context(tc.tile_pool(name="ptr", bufs=1, space="PSUM"))
    pacc = ctx.enter_context(tc.tile_pool(name="pacc", bufs=1, space="PSUM"))
    pout = ctx.enter_context(tc.tile_pool(name="pout", bufs=1, space="PSUM"))

    # ---- constants / small loads ----
    ident128 = sb.tile([128, 128], f32, tag="ident128")
    make_identity(nc, ident128)
    ident4 = sb.tile([4, 4], f32, tag="ident4")
    make_identity(nc, ident4)
    ident1 = sb.tile([1, 1], f32, tag="ident1")
    make_identity(nc, ident1)

    iota_p = sb.tile([128, 1], u32, tag="iota_p")
    nc.gpsimd.iota(iota_p, pattern=[[0, 1]], base=0, channel_multiplier=1)
    sub64 = sb.tile([128, 1], u32, tag="sub64")
    nc.vector.memset(sub64[0:64, :], 0)
    nc.vector.memset(sub64[64:128, :], 64)
    l_mod = sb.tile([128, 1], u32, tag="l_mod")  # p % 64
    nc.vector.tensor_tensor(
        out=l_mod, in0=iota_p, in1=sub64, op=mybir.AluOpType.subtract
    )

    # ---- x loads; xt (transposed x chunks) built on the PE ----
    x_nat = sb.tile([BS, E], f32, tag="x_nat")
    nc.sync.dma_start(out=x_nat, in_=x_d.ap())
    xrows = []
    for b in range(BS):
        xr = sb.tile([1, E], f32, tag=f"xr{b}")
        nc.sync.dma_start(out=xr, in_=x_d.ap()[b : b + 1, :])
        xrows.append(xr)

    psum_xt = ptr.tile([128, NC_CHUNKS * BS], f32, tag="tr", space="PSUM")
    for c in range(NC_CHUNKS):
        nc.tensor.transpose(
            out=psum_xt[:, c * BS : (c + 1) * BS],
            in_=x_nat[:, c * 128 : (c + 1) * 128],
            identity=ident4,
        )
    xt = sb.tile([128, NC_CHUNKS, BS], f32r, tag="xt")
    nc.vector.tensor_copy(out=xt.rearrange("p c b -> p (c b)"), in_=psum_xt)

    bc_sb = sb.tile([1, H], f32, tag="bc_sb")
    nc.sync.dma_start(out=bc_sb, in_=bc_d.ap())
    bc_b = sb.tile([BS, H], f32, tag="bc_b")
    nc.gpsimd.partition_broadcast(bc_b, bc_sb[0:1, :])

    qkb_sb = sb.tile([1, H], f32, tag="qkb_sb")
    nc.sync.dma_start(out=qkb_sb, in_=qkb_d.ap())
    qkb_b = sb.tile([BS, H], f32, tag="qkb_b")
    nc.gpsimd.partition_broadcast(qkb_b, qkb_sb[0:1, :])

    # ---- cache-key loads (own queue: vector) ----
    ktiles = []
    for b in range(BS):
        kb = kpool.tile([128, E], f32, tag="kb")
        nc.sync.dma_start(out=kb, in_=keys_d.ap()[b])
        ktiles.append(kb)

    # ---- scores via DVE mult+reduce against broadcast x ----
    scoresP = sb.tile([128, BS], f32, tag="scoresP")
    for b in range(BS):
        xb = bpool.tile([128, E], f32, tag="bc128")
        nc.gpsimd.partition_broadcast(xb, xrows[b])
        scr = scpool.tile([128, E], f32, tag="scr")
        nc.vector.tensor_tensor(
            out=scr, in0=ktiles[b], in1=xb, op=mybir.AluOpType.mult
        )
        nc.vector.tensor_reduce(
            out=scoresP[:, b : b + 1],
            in_=scr,
            axis=mybir.AxisListType.X,
            op=mybir.AluOpType.add,
        )

    # ---- final projection, x-half (starts as soon as wct streams) ----
    psum_out = pout.tile([BS, H], f32, tag="psum_out", space="PSUM")
    for j in range(NC_CHUNKS):
        wch = wpool.tile([128, H], f32r, tag="w")
        eng = nc.sync if j % 2 == 0 else nc.scalar
        eng.dma_start(out=wch, in_=wct_d.ap()[H + j * 128 : H + (j + 1) * 128, :])
        for hh in range(2):
            nc.tensor.matmul(
                out=psum_out[:, hh * 512 : (hh + 1) * 512],
                lhsT=xt[:, j, :],
                rhs=wch[:, hh * 512 : (hh + 1) * 512],
                start=(j == 0),
                stop=False,
                skip_group_check=True,
            )

    # ---- qk = (x @ M + qkb) / sqrt(H)  (scale folded into M on host) ----
    psum_qk = pacc.tile([BS, H], f32, tag="acc", space="PSUM")
    for c in range(NC_CHUNKS):
        mch = wpool.tile([128, H], f32r, tag="w")
        eng = nc.sync if c % 2 == 0 else nc.scalar
        eng.dma_start(out=mch, in_=m_d.ap()[c * 128 : (c + 1) * 128, :])
        for hh in range(2):
            nc.tensor.matmul(
                out=psum_qk[:, hh * 512 : (hh + 1) * 512],
                lhsT=xt[:, c, :],
                rhs=mch[:, hh * 512 : (hh + 1) * 512],
                start=(c == 0),
                stop=(c == NC_CHUNKS - 1),
            )
    qk_sb = sb.tile([BS, H], f32, tag="qk_sb")
    nc.vector.tensor_tensor(
        out=qk_sb, in0=psum_qk, in1=qkb_b, op=mybir.AluOpType.add
    )

    # ---- transpose scores -> [BS, N]; top-8 ----
    psum_tr = ptr.tile([BS, 128], f32, tag="tr", space="PSUM")
    nc.tensor.transpose(out=psum_tr, in_=scoresP, identity=ident128)
    scoresF = sb.tile([BS, N], f32, tag="scoresF")
    nc.vector.tensor_copy(out=scoresF, in_=psum_tr)

    top8 = sb.tile([BS, 8], f32, tag="top8")
    topi = sb.tile([BS, 8], u32, tag="topi")
    nc.vector.max_with_indices(out_max=top8, out_indices=topi, in_=scoresF)

    negmax = sb.tile([BS, 1], f32, tag="negmax")
    nc.vector.tensor_scalar_mul(negmax, top8[:, 0:1], -1.0)
    e8 = sb.tile([BS, 8], f32, tag="e8")
    sum8 = sb.tile([BS, 1], f32, tag="sum8")
    nc.scalar.activation(
        out=e8,
        in_=top8,
        func=mybir.ActivationFunctionType.Exp,
        bias=negmax,
        scale=1.0,
        accum_out=sum8,
    )

    # ---- gather offsets + indirect zone gathers ----
    offs = []
    for b in range(BS):
        trow = sb.tile([1, 8], u32, tag=f"trow{b}")
        nc.sync.dma_start(out=trow, in_=topi[b : b + 1, :])
        t8u = bpool.tile([128, 8], u32, tag="t8u")
        nc.gpsimd.partition_broadcast(t8u, trow)
        t8v = t8u.rearrange("p (k two) -> p k two", two=2)
        off_b = sb.tile([128, ZCH], u32, tag=f"off{b}")
        nc.vector.tensor_copy(out=off_b[0:64, :], in_=t8v[0:64, :, 0])
        nc.vector.tensor_copy(out=off_b[64:128, :], in_=t8v[64:128, :, 1])
        nc.vector.tensor_scalar(
            out=off_b,
            in0=off_b,
            scalar1=L,  # slot index n -> row n*L
            scalar2=b * N * L,  # batch-row base in vals_flat
            op0=mybir.AluOpType.mult,
            op1=mybir.AluOpType.add,
        )
        nc.vector.tensor_tensor(
            out=off_b,
            in0=off_b,
            in1=l_mod.to_broadcast([128, ZCH]),
            op=mybir.AluOpType.add,
        )
        offs.append(off_b)

    ztiles = {}
    for b in range(BS):
        for c in range(ZCH):
            z = zpool.tile([128, H], f32r, tag=f"z{b}_{c}")
            nc.gpsimd.indirect_dma_start(
                out=z,
                out_offset=None,
                in_=vals_flat,
                in_offset=bass.IndirectOffsetOnAxis(ap=offs[b][:, c : c + 1], axis=0),
            )
            ztiles[(b, c)] = z

    # ---- logits via DVE mult+reduce against broadcast qk ----
    logitsP = sb.tile([128, BS * ZCH], f32, tag="logitsP")
    for b in range(BS):
        qrow = sb.tile([1, H], f32, tag=f"qrow{b % 2}")
        nc.sync.dma_start(out=qrow, in_=qk_sb[b : b + 1, :])
        qkb_bc = bpool.tile([128, H], f32, tag="bc128")
        nc.gpsimd.partition_broadcast(qkb_bc, qrow)
        for c in range(ZCH):
            scr2 = scpool.tile([128, H], f32, tag="scr")
            nc.vector.tensor_tensor(
                out=scr2,
                in0=ztiles[(b, c)].bitcast(f32),
                in1=qkb_bc,
                op=mybir.AluOpType.mult,
            )
            nc.vector.tensor_reduce(
                out=logitsP[:, b * ZCH + c : b * ZCH + c + 1],
                in_=scr2,
                axis=mybir.AxisListType.X,
                op=mybir.AluOpType.add,
            )

    # ---- logits [128, b*4+c] -> [BS, KL] via 4 strided-column transposes ----
    lgt = sb.tile([BS, KL], f32, tag="lgt")
    psum_lt = ptr.tile([BS, KL], f32, tag="lt", space="PSUM")
    for c in range(ZCH):
        # columns c, 4+c, 8+c, 12+c = chunk c of b=0..3 -> rows 0..3
        nc.tensor.transpose(
            out=psum_lt[:, c * 128 : (c + 1) * 128],
            in_=logitsP.rearrange("p (b c) -> p c b", c=ZCH)[:, c, :],
            identity=ident128,
        )
    nc.vector.tensor_copy(out=lgt, in_=psum_lt)

    # ---- zone softmax + combined coefficients ----
    lgt_v = lgt.rearrange("b (k l) -> b k l", k=K)
    zmax = sb.tile([BS, K], f32, tag="zmax")
    nc.vector.tensor_reduce(
        out=zmax, in_=lgt_v, axis=mybir.AxisListType.X, op=mybir.AluOpType.max
    )
    lc = sb.tile([BS, KL], f32, tag="lc")
    nc.vector.tensor_tensor(
        out=lc.rearrange("b (k l) -> b k l", k=K),
        in0=lgt_v,
        in1=zmax[:, :, None].to_broadcast([BS, K, L]),
        op=mybir.AluOpType.subtract,
    )
    ez = sb.tile([BS, KL], f32, tag="ez")
    nc.scalar.activation(out=ez, in_=lc, func=mybir.ActivationFunctionType.Exp)
    zsum = sb.tile([BS, K], f32, tag="zsum")
    nc.vector.tensor_reduce(
        out=zsum,
        in_=ez.rearrange("b (k l) -> b k l", k=K),
        axis=mybir.AxisListType.X,
        op=mybir.AluOpType.add,
    )
    t8s = sb.tile([BS, K], f32, tag="t8s")
    nc.vector.tensor_scalar_mul(t8s, zsum, sum8)
    rec8 = sb.tile([BS, K], f32, tag="rec8")
    nc.vector.reciprocal(rec8, t8s)
    s8 = sb.tile([BS, K], f32, tag="s8")
    nc.vector.tensor_tensor(out=s8, in0=e8, in1=rec8, op=mybir.AluOpType.mult)
    coeff = sb.tile([BS, KL], f32, tag="coeff")
    nc.vector.tensor_tensor(
        out=coeff.rearrange("b (k l) -> b k l", k=K),
        in0=ez.rearrange("b (k l) -> b k l", k=K),
        in1=s8[:, :, None].to_broadcast([BS, K, L]),
        op=mybir.AluOpType.mult,
    )

    # ---- transpose coeff -> [128, ZCH*BS] column layout ----
    psum_ct = ptr.tile([128, ZCH * BS], f32, tag="tr", space="PSUM")
    for c in range(ZCH):
        nc.tensor.transpose(
            out=psum_ct[:, c * BS : (c + 1) * BS],
            in_=coeff[:, c * 128 : (c + 1) * 128],
            identity=ident4,
        )
    ctT = sb.tile([128, ZCH * BS], f32r, tag="ctT")
    nc.vector.tensor_copy(out=ctT, in_=psum_ct)

    # ---- attn_out[b] = coeff_b.T @ Z_b; transpose rows on PE ----
    attT = sb.tile([128, NC_CHUNKS, BS], f32r, tag="attT")
    for b in range(BS):
        psum_att = pacc.tile([1, H], f32, tag="acc", space="PSUM")
        for c in range(ZCH):
            for hh in range(2):
                nc.tensor.matmul(
                    out=psum_att[:, hh * 512 : (hh + 1) * 512],
                    lhsT=ctT[:, c * BS + b : c * BS + b + 1],
                    rhs=ztiles[(b, c)][:, hh * 512 : (hh + 1) * 512],
                    start=(c == 0),
                    stop=(c == ZCH - 1),
                )
        att_sb = sb.tile([1, H], f32, tag=f"att_sb{b % 2}")
        nc.scalar.copy(out=att_sb, in_=psum_att)
        psum_at = ptr.tile([128, NC_CHUNKS], f32, tag="at", space="PSUM", bufs=2)
        for c in range(NC_CHUNKS):
            nc.tensor.transpose(
                out=psum_at[:, c : c + 1],
                in_=att_sb[:, c * 128 : (c + 1) * 128],
                identity=ident1,
            )
        nc.vector.tensor_copy(out=attT[:, :, b], in_=psum_at)

    # ---- final projection, attn half; bias; store ----
    for j in range(NC_CHUNKS):
        wch = wpool.tile([128, H], f32r, tag="w")
        eng = nc.sync if j % 2 == 0 else nc.scalar
        eng.dma_start(out=wch, in_=wct_d.ap()[j * 128 : (j + 1) * 128, :])
        for hh in range(2):
            nc.tensor.matmul(
                out=psum_out[:, hh * 512 : (hh + 1) * 512],
                lhsT=attT[:, j, :],
                rhs=wch[:, hh * 512 : (hh + 1) * 512],
                start=False,
                stop=(j == NC_CHUNKS - 1),
                skip_group_check=True,
            )
    out_sb = sb.tile([BS, H], f32, tag="out_sb")
    nc.vector.tensor_tensor(
        out=out_sb, in0=psum_out, in1=bc_b, op=mybir.AluOpType.add
    )
    nc.sync.dma_start(out=out_d.ap(), in_=out_sb)
    ctx.close()


def build_program():
    import concourse.bacc as bacc
    import concourse.tile as tile

    nc = bacc.Bacc("TRN2", target_bir_lowering=False, debug=False, enable_asserts=False)
    with tile.TileContext(nc) as tc:
        emit(nc, tc)
    nc.compile()
    return nc


def host_prep(inputs, Wq, bq, Wk, bk, Wc, bc, cache_keys, cache_values):
    """Host-side weight folding + per-core input shards."""
    x = np.asarray(inputs, dtype=np.float32)
    Wq = np.asarray(Wq, dtype=np.float32)
    Wk = np.asarray(Wk, dtype=np.float32)
    Wc = np.asarray(Wc, dtype=np.float32)
    # fold the attention 1/sqrt(H) into the query projection
    m_w = np.ascontiguousarray((Wq.T @ Wk) * INV_SQRT_H, dtype=np.float32)  # [E, H]
    qkb = np.ascontiguousarray(
        ((np.asarray(bq, np.float32) @ Wk) * INV_SQRT_H).reshape(1, H),
        dtype=np.float32,
    )
    wct = np.ascontiguousarray(Wc.T, dtype=np.float32)  # [2H, H]
    bc2 = np.ascontiguousarray(np.asarray(bc, np.float32).reshape(1, H))
    keys = np.asarray(cache_keys, dtype=np.float32)
    vals = np.asarray(cache_values, dtype=np.float32)

    in_maps = []
    for c in range(NCORES):
        sl = slice(c * BS, (c + 1) * BS)
        in_maps.append(
            {
                "x_sh": np.ascontiguousarray(x[sl]),
                "keys_sh": np.ascontiguousarray(keys[sl]),
                "vals_sh": np.ascontiguousarray(vals[sl]),
                "m_w": m_w,
                "qkb": qkb,
                "wct": wct,
                "bc": bc2,
            }
        )
    return in_maps


_CACHED = {}


def kernel(**inputs):
    from concourse.bass_utils import run_bass_kernel_spmd

    in_maps = host_prep(**inputs)
    if "nc" not in _CACHED:
        _CACHED["nc"] = build_program()
    nc = _CACHED["nc"]
    res = run_bass_kernel_spmd(nc, in_maps, core_ids=list(range(NCORES)))
    _CACHED["last_res"] = res
    out = np.concatenate([r["out_sh"] for r in res.results], axis=0)
    return out.astype(np.float32)


if __name__ == "__main__":
    import reference

    ins = {k: np.asarray(v) for k, v in reference.setup_inputs().items()}
    got = kernel(**ins)
    exp = np.asarray(reference.reference(**ins))
    err = np.abs(got - exp).max() / (np.abs(exp).max() + 1e-12)
    print("Relative error:", err)
